# revision 1
# baseline (speedup 1.0000x reference)
"""Block-sparse linear (BSR 32x32 blocks) on 8 Trainium2 NeuronCores.

y = x @ W^T + bias, W given as BSR blocks (nnz, 32, 32) * mask, 128 row
blocks x 128 col blocks, 13 nnz col blocks per row block.

Strategy (data-parallel over batch):
  - Shard x [2048, 4096] by batch across 8 cores -> [256, 4096] per core.
  - Host packs per-core x^T into a blocked SBUF-friendly layout
    xt[128, 32*256]: col block c lives at partitions g(c)*32..+32,
    free columns chunk(c)*256..+256 (g/chunk chosen to load-balance the
    4 partition groups by block count).
  - Weights (mask*weight, replicated) packed as lhsT strips
    wt[128, n_slots*32]: block j at partitions g*32..+32, cols slot_j*32..+32,
    strip = w_block.T (so matmul computes w_block @ x_c^T).
  - Device: per block one K=32, M=32, N=256 matmul accumulated into a
    PSUM tile [128, 256] holding 4 row blocks (PE 32x32 array tiling ->
    up to 16 concurrent block matmuls). Evacuate with bias add
    (ScalarE/VectorE alternating), DMA y^T back in blocked layout.
  - Host unblocks y^T -> y.
"""

import os
import sys

import numpy as np

sys.path.insert(0, "/opt/trn_rl_repo")

import concourse.bass as bass
import concourse.mybir as mybir
from concourse import bacc
from concourse.bass_utils import run_bass_kernel_spmd
from concourse.tile import TileContext

BS = 32          # block size
RB = 128         # row blocks
CB = 128         # col blocks
BATCH = 2048
N_CORES = 8
BPC = BATCH // N_CORES   # batch per core = 256
N_CHUNKS = CB // 4       # 32 chunks of 4 col blocks in xt layout
N_SUPER = RB // 4        # 32 super-iterations (4 row blocks each)
OUT_GROUP = 4            # super-iters per output DMA

# matmul input dtype: "f32" (exact, rel err ~1e-7, 4 cyc/row on the PE)
# or "bf16" (rel err ~3e-3 vs the f32 reference under the harness's 2e-2
# relative-error gate; halves DMA traffic and runs 1 cyc/row on the PE).
# f32r is NOT usable here: TRN2 rejects fp32r matmuls with a non-zero dst
# partition, which this kernel's 32x32 column tiling needs.
MM_DTYPE = os.environ.get("BSL_MM_DTYPE", "bf16")

_CACHE = {}


def _layout(crow, col):
    """Choose (group, chunk) for each col block + build the MM schedule.

    PSUM hazard rule (HW-fatal otherwise): two concurrent matmuls from
    different PE row-groups must never write the same PSUM bank.  So each
    super-iter keeps FOUR psum tiles (banks), one per row-group g; the
    chain of blocks of row r with col-group g accumulates serially on PE
    tile (g, rl) into psum P_g partitions rl*32..+32.  The 4 partials are
    summed on ScalarE/VectorE at evacuation.
    """
    deg = np.bincount(col, minlength=CB)
    # balance total block count across the 4 partition groups, 32 cols each
    order = np.argsort(-deg, kind="stable")
    load = [0] * 4
    cnt = [0] * 4
    gof = np.zeros(CB, np.int64)
    for c in order:
        cands = [g for g in range(4) if cnt[g] < N_CHUNKS]
        g = min(cands, key=lambda gi: load[gi])
        gof[c] = g
        load[g] += int(deg[c])
        cnt[g] += 1
    chunkof = np.zeros(CB, np.int64)
    nxt = [0] * 4
    for c in range(CB):
        g = gof[c]
        chunkof[c] = nxt[g]
        nxt[g] += 1

    # chains[r][g] = list of (chunk, widx) for row block r, col-group g
    chains = []
    for r in range(RB):
        per_g = [[] for _ in range(4)]
        for j in range(int(crow[r]), int(crow[r + 1])):
            c = int(col[j])
            per_g[int(gof[c])].append((int(chunkof[c]), j))
        chains.append(per_g)

    # a chunk belonging to each group, for dummy (zero-weight) matmuls
    g_any_chunk = []
    for g in range(4):
        for c in range(CB):
            if gof[c] == g:
                g_any_chunk.append(int(chunkof[c]))
                break

    # Per row: bias matmul (widx=-2, K=1) goes on the least-loaded group;
    # remaining empty groups get a dummy zero-weight matmul (widx=-1) so
    # every psum region is written.
    full = []
    loads = np.zeros((RB, 4), np.int64)
    for r in range(RB):
        per_g = [list(chains[r][g]) for g in range(4)]
        gb = int(np.argmin([len(c) for c in per_g]))
        per_g[gb] = [(-gb - 1, -2)] + per_g[gb]  # chunk field stores -(g+1)
        for g in range(4):
            if not per_g[g]:
                per_g[g] = [(g_any_chunk[g], -1)]
            loads[r, g] = len(per_g[g])
        full.append(per_g)

    # Pair rows so per-tile loads (l_A+l_B per group) are balanced, then
    # chunk pairs into pair-slots by descending max so one long chain
    # doesn't straggle a pair-slot of short ones.
    rows_left = sorted(range(RB), key=lambda r: -int(loads[r].max()))
    pairs = []
    while rows_left:
        a = rows_left.pop(0)
        best_j = min(
            range(len(rows_left)),
            key=lambda j: int((loads[a] + loads[rows_left[j]]).max()),
        )
        b = rows_left.pop(best_j)
        pairs.append((a, b))
    pairs.sort(key=lambda ab: -int((loads[ab[0]] + loads[ab[1]]).max()))
    # slot_rows[s][rl] = actual row block handled by super-iter s, lane rl
    slot_rows = [[0] * 4 for _ in range(N_SUPER)]
    for p in range(N_SUPER // 2):
        for rl in range(4):
            a, b = pairs[4 * p + rl]
            slot_rows[2 * p][rl] = a
            slot_rows[2 * p + 1][rl] = b

    slot_counter = [1] * 4
    sched = []  # per pair: list of (s, rl, g, chunk, slot, widx, start, stop)
    for p in range(N_SUPER // 2):
        todo = {}
        for s in (2 * p, 2 * p + 1):
            for rl in range(4):
                r = slot_rows[s][rl]
                for g in range(4):
                    todo[(s, rl, g)] = list(full[r][g])
        # merge the two super-iters' chains per PE tile (rl, g): one
        # accumulation group per (tile, pair) because start=True clears
        # has_written for the whole bank column range at those partitions
        merged = {}
        for rl in range(4):
            for g in range(4):
                a = [(2 * p, c, w) for (c, w) in todo[(2 * p, rl, g)]]
                b = [(2 * p + 1, c, w) for (c, w) in todo[(2 * p + 1, rl, g)]]
                m = []
                for k in range(max(len(a), len(b))):
                    if k < len(a):
                        m.append(a[k])
                    if k < len(b):
                        m.append(b[k])
                merged[(rl, g)] = m
        max_len = max(len(v) for v in merged.values())
        items = []
        for k in range(max_len):
            for rl in range(4):
                for g in range(4):
                    ch = merged[(rl, g)]
                    if k < len(ch):
                        s, chunk, widx = ch[k]
                        if widx < 0:
                            slot = 0
                        else:
                            slot = slot_counter[g]
                            slot_counter[g] += 1
                        items.append(
                            (s, rl, g, chunk, slot, widx, k == 0, k == len(ch) - 1)
                        )
        sched.append(items)
    n_slots = max(slot_counter)
    return gof, chunkof, sched, n_slots, slot_rows


def _mm_dt():
    if MM_DTYPE == "bf16":
        return mybir.dt.bfloat16
    if MM_DTYPE == "f32r":
        return mybir.dt.float32r
    return mybir.dt.float32


def _np_dt():
    if MM_DTYPE == "bf16":
        import ml_dtypes

        return ml_dtypes.bfloat16
    return np.float32


def _build_program(sched, n_slots, slot_rows, reps=1, ablate=""):
    nc = bacc.Bacc(
        "TRN2",
        target_bir_lowering=False,
        debug=False,
        num_devices=N_CORES,
    )
    dt_in = _mm_dt()
    f32 = mybir.dt.float32

    xt = nc.dram_tensor("xt", [128, N_CHUNKS * BPC], dt_in, kind="ExternalInput")
    wt = nc.dram_tensor("wt", [128, n_slots * BS], dt_in, kind="ExternalInput")
    bias_d = nc.dram_tensor("bias_d", [4, RB * BS], dt_in, kind="ExternalInput")
    # Output rides in the matmul dtype (bf16 when MM_DTYPE=bf16): the host
    # upcasts after gathering.  Halves the output DMA traffic.
    dt_out = dt_in if MM_DTYPE == "bf16" else f32
    yt = nc.dram_tensor("yt", [128, N_SUPER * BPC], dt_out, kind="ExternalOutput")

    with TileContext(nc) as tc:
        with (
            tc.tile_pool(name="persist", bufs=(2 if reps > 1 else 1)) as persist,
            tc.tile_pool(name="psum", bufs=2, space="PSUM") as psum_pool,
            tc.tile_pool(name="stage", bufs=3) as stage_pool,
            tc.tile_pool(name="tmp", bufs=2) as tmp_pool,
        ):
          for _rep in range(reps):
            if True:
                xt_sb = persist.tile(
                    [128, N_CHUNKS * BPC], dt_in, tag="xt_sb", name="xt_sb"
                )
                nc.sync.dma_start(out=xt_sb, in_=xt[:])
                bias_sb = persist.tile(
                    [128, RB * BS], dt_in, tag="bias_sb", name="bias_sb"
                )
                for g in range(4):
                    nc.sync.dma_start(
                        out=bias_sb[g * BS : g * BS + 1, :], in_=bias_d[g : g + 1, :]
                    )
                ones_sb = persist.tile([128, BPC], dt_in, tag="ones_sb", name="ones_sb")
                nc.vector.memset(ones_sb, 1.0)
                # stream weights in pieces (separate tiles -> per-piece deps)
                n_pieces = 8
                sp = -(-n_slots // n_pieces)
                wt_pieces = []
                for p in range(n_pieces):
                    lo = p * sp * BS
                    hi = min((p + 1) * sp * BS, n_slots * BS)
                    if lo >= hi:
                        break
                    wp = persist.tile(
                        [128, hi - lo], dt_in, tag=f"wt_sb{p}", name=f"wt_sb{p}"
                    )
                    nc.sync.dma_start(out=wp, in_=wt[:, lo:hi])
                    wt_pieces.append(wp)

            stage = None
            pending = None

            def emit_evac(p, P):
                nonlocal stage
                if p % 2 == 0:
                    stage = stage_pool.tile(
                        [128, OUT_GROUP * BPC], dt_out, tag="st", name=f"st_{p}"
                    )
                dst = stage[:, (p % 2) * 2 * BPC : (p % 2 + 1) * 2 * BPC]
                e0 = tmp_pool.tile([128, 2 * BPC], dt_out, tag="e0", name=f"e0_{p}")
                e1 = tmp_pool.tile([128, 2 * BPC], dt_out, tag="e1", name=f"e1_{p}")
                e2 = tmp_pool.tile([128, 2 * BPC], dt_out, tag="e2", name=f"e2_{p}")
                d0 = tmp_pool.tile([128, 2 * BPC], dt_out, tag="d0", name=f"d0_{p}")
                d1 = tmp_pool.tile([128, 2 * BPC], dt_out, tag="d1", name=f"d1_{p}")
                if "evac1" in ablate:
                    nc.scalar.copy(dst, P[0][:, :])
                    nc.scalar.copy(e0, P[1][:, :])
                    nc.vector.tensor_copy(e2, P[2][:, :])
                    nc.vector.tensor_copy(d0, P[3][:, :])
                else:
                    # PSUM extraction split: ACT reads 3 banks (2x mode),
                    # DVE reads 1 bank (TT from PSUM is 1x) + one bf16 SBUF
                    # add (2x), GPSIMD does the final SBUF-only add.  Keeps
                    # every engine's per-pair cost near/under ~1.1us.
                    nc.scalar.copy(e0, P[0][:, :])
                    nc.scalar.copy(e1, P[1][:, :])
                    nc.scalar.copy(e2, P[2][:, :])
                    nc.vector.tensor_add(d0, e0, e1)
                    nc.vector.tensor_add(d1, e2, P[3][:, :])
                    nc.gpsimd.tensor_add(dst, d0, d1)
                if p % 2 == 1:
                    # scalar-engine HWDGE ring: keeps evac-gated output DMAs
                    # from head-of-line-blocking the SP input-DMA ring
                    nc.scalar.dma_start(
                        out=yt[:, 2 * (p - 1) * BPC : (2 * (p - 1) + 4) * BPC],
                        in_=stage,
                    )

            for p, items in enumerate(sched):
                psums = [
                    psum_pool.tile(
                        [128, 2 * BPC], f32, tag=f"acc{g}", name=f"acc{g}_{p}"
                    )
                    for g in range(4)
                ]
                for (s, rl, g, chunk, slot, widx, start, stop) in items:
                    if "no_mm" in ablate:
                        break
                    off = (s % 2) * BPC
                    dst_ap = psums[g][rl * BS : (rl + 1) * BS, off : off + BPC]
                    if widx == -2:
                        r = slot_rows[s][rl]
                        nc.tensor.matmul(
                            out=dst_ap,
                            lhsT=bias_sb[
                                g * BS : g * BS + 1, r * BS : (r + 1) * BS
                            ],
                            rhs=ones_sb[g * BS : g * BS + 1, :],
                            start=start,
                            stop=stop,
                            tile_position=(g * BS, rl * BS),
                        )
                        continue
                    pc, ps = divmod(slot, sp)
                    nc.tensor.matmul(
                        out=dst_ap,
                        lhsT=wt_pieces[pc][
                            g * BS : (g + 1) * BS, ps * BS : (ps + 1) * BS
                        ],
                        rhs=xt_sb[
                            g * BS : (g + 1) * BS, chunk * BPC : (chunk + 1) * BPC
                        ],
                        start=start,
                        stop=stop,
                        tile_position=(g * BS, rl * BS),
                    )
                if "no_evac" in ablate:
                    continue
                if pending is not None:
                    emit_evac(*pending)
                pending = (p, psums)
            if pending is not None:
                emit_evac(*pending)
            pending = None
            if "no_evac" in ablate:
                # still produce the output bytes so DMA-out traffic is equal
                stq = stage_pool.tile(
                    [128, OUT_GROUP * BPC], dt_out, tag="st", name="stq"
                )
                nc.vector.memset(stq, 0.0)
                for q in range(0, N_SUPER, OUT_GROUP):
                    nc.scalar.dma_start(
                        out=yt[:, q * BPC : (q + OUT_GROUP) * BPC], in_=stq
                    )
    nc.compile()
    return nc


def _pack_inputs(x, gof, chunkof, sched, n_slots, w, bias):
    np_in = _np_dt()
    # xt per core
    xts = []
    for core in range(N_CORES):
        xc = x[core * BPC : (core + 1) * BPC]  # [256, 4096]
        xt_in = np.zeros((128, N_CHUNKS * BPC), np.float32)
        for c in range(CB):
            g = int(gof[c])
            ch = int(chunkof[c])
            xt_in[g * BS : (g + 1) * BS, ch * BPC : (ch + 1) * BPC] = xc[
                :, c * BS : (c + 1) * BS
            ].T
        xts.append(np.ascontiguousarray(xt_in.astype(np_in)))
    # weights (slot 0 per group stays all-zero for dummy matmuls)
    wt_in = np.zeros((128, n_slots * BS), np.float32)
    for items in sched:
        for (_s, _rl, g, _chunk, slot, widx, _a, _b) in items:
            if widx >= 0:
                wt_in[g * BS : (g + 1) * BS, slot * BS : (slot + 1) * BS] = w[widx].T
    wt_in = np.ascontiguousarray(wt_in.astype(np_in))
    # bias rows for the K=1 bias matmuls, one copy per partition group
    bias_in = np.ascontiguousarray(
        np.broadcast_to(bias.reshape(1, RB * BS), (4, RB * BS)).astype(np_in)
    )
    return xts, wt_in, bias_in


def kernel(x, crow_indices, col_indices, mask, weight, bias):
    x = np.asarray(x, np.float32)
    crow = np.asarray(crow_indices, np.int64)
    col = np.asarray(col_indices, np.int64)
    w = (np.asarray(mask, np.float32) * np.asarray(weight, np.float32))
    bias_np = np.asarray(bias, np.float32)

    key = (crow.tobytes(), col.tobytes(), MM_DTYPE)
    if key not in _CACHE:
        gof, chunkof, sched, n_slots, slot_rows = _layout(crow, col)
        nc = _build_program(sched, n_slots, slot_rows)
        _CACHE.clear()
        _CACHE[key] = (gof, chunkof, sched, n_slots, slot_rows, nc)
    gof, chunkof, sched, n_slots, slot_rows, nc = _CACHE[key]

    xts, wt_in, bias_in = _pack_inputs(x, gof, chunkof, sched, n_slots, w, bias_np)
    in_maps = [
        {"xt": xts[core], "wt": wt_in, "bias_d": bias_in} for core in range(N_CORES)
    ]
    res = run_bass_kernel_spmd(nc, in_maps, core_ids=list(range(N_CORES)))

    outs = []
    for core in range(N_CORES):
        ytc = np.asarray(res.results[core]["yt"]).astype(np.float32)
        yc = np.empty((BPC, RB * BS), np.float32)
        for s in range(N_SUPER):
            for rl in range(4):
                r = slot_rows[s][rl]
                yc[:, r * BS : (r + 1) * BS] = ytc[
                    rl * BS : (rl + 1) * BS, s * BPC : (s + 1) * BPC
                ].T
        outs.append(yc)
    return np.ascontiguousarray(np.concatenate(outs, axis=0))



# revision 8
# speedup vs baseline: 4.4060x; 4.4060x over previous
"""Block-sparse linear (BSR 32x32) on 8 NeuronCores: 4-way batch x 2-way
row sharding, N=512 matmuls.

Per-instruction PE cost (~14.5 ns) dominates this kernel, so the win over
v1 is halving instructions per core: batch quarters (BPC=512) double the
moving dim per matmul at equal per-core DMA, and row halves (64 row blocks
per core) halve the block count per core.

Cores 0-3: batch quarter q = core, row blocks 0-63  (program A)
Cores 4-7: batch quarter q = core-4, row blocks 64-127 (program B)
Host assembles y from the two halves.  A and B have different sparsity ->
two compiled programs, dispatched as two 4-core SPMD jobs.
"""

import sys

import numpy as np

sys.path.insert(0, "/opt/trn_rl_repo")

import concourse.mybir as mybir
from concourse import bacc
from concourse.bass_utils import run_bass_kernel_spmd
from concourse.tile import TileContext

BS = 32
RB = 128
CB = 128
BATCH = 2048
N_CORES = 8
BQ = 512          # batch per core (quarter)
RBH = 64          # row blocks per half
N_CHUNKS = CB // 4
N_QUADS = RBH // 4

_CACHE = {}


def _strip_mm_incs(nc):
    """See kernel.py: drop per-matmul sem increments nobody waits on."""
    import concourse.mybir as mb

    blocks = nc.m.functions[0].blocks
    all_insts = [i for blk in blocks for i in blk.instructions]
    mms = [
        i
        for i in all_insts
        if isinstance(i, mb.InstMatmult) and i.sync_info is not None
    ]
    sem_ids = set()
    for m in mms:
        for u in m.sync_info.on_update:
            sem_ids.add(u.id)
    if not sem_ids:
        return
    assert len(sem_ids) == 1, f"multiple PE sems: {sem_ids}"
    sem = sem_ids.pop()
    for inst in all_insts:
        if isinstance(inst, mb.InstMatmult) or inst.sync_info is None:
            continue
        for u in inst.sync_info.on_update:
            assert u.id != sem, f"non-MM update to PE sem by {inst.name}"
    waits = []
    for inst in all_insts:
        si = inst.sync_info
        if si is None:
            continue
        for wi, w in enumerate(si.on_wait):
            if w.id == sem:
                assert w.wait_mode == "sem-ge-imm", w
                waits.append((inst, wi, w.wait_value))
    thresholds = sorted({v for (_, _, v) in waits})
    rank = {v: i + 1 for i, v in enumerate(thresholds)}
    keep = set(thresholds)
    k = 0
    for m in mms:
        k += 1
        if k not in keep:
            m.sync_info = mb.SyncInfo(
                on_wait=list(m.sync_info.on_wait), on_update=[]
            )
    for inst, wi, v in waits:
        si = inst.sync_info
        new_waits = list(si.on_wait)
        w = new_waits[wi]
        new_waits[wi] = mb.SyncWait(
            sync_type=w.sync_type,
            id=w.id,
            ant_name=w.ant_name,
            wait_mode=w.wait_mode,
            wait_value=rank[v],
            wait_reg=None,
        )
        inst.sync_info = mb.SyncInfo(
            on_wait=new_waits, on_update=list(si.on_update)
        )


def _layout_half(crow, col, rows):
    """Layout for one row half: col -> (group, chunk), chains, quad schedule.

    Same PSUM discipline as v1: per quad, four psum banks (one per
    partition group g); the chain of row r's blocks with col-group g
    accumulates serially on PE tile (g, rl).

    Two passes: pass 1 assigns groups by degree to balance lanes and picks
    quads; pass 2 re-numbers chunks in FIRST-USE order (by quad) so early
    quads depend only on early xt pieces and the PE starts while xt is
    still streaming.  Bias rides the evacuation (ACT bias operand), not
    matmuls; empty (row, group) chains get a zero-weight dummy matmul so
    every psum region is written.
    """
    cols_of = {int(r): col[int(crow[r]) : int(crow[r + 1])].astype(int) for r in rows}
    deg = np.zeros(CB, np.int64)
    for r in rows:
        for c in cols_of[int(r)]:
            deg[c] += 1

    def assign_groups(order):
        load = [0] * 4
        cnt = [0] * 4
        gof = np.zeros(CB, np.int64)
        chunkof = np.zeros(CB, np.int64)
        for c in order:
            cands = [g for g in range(4) if cnt[g] < N_CHUNKS]
            g = min(cands, key=lambda gi: load[gi])
            gof[c] = g
            chunkof[c] = cnt[g]
            load[g] += int(deg[c])
            cnt[g] += 1
        return gof, chunkof

    def build_chains(gof, chunkof):
        chains = {}
        for r in rows:
            r = int(r)
            per_g = [[] for _ in range(4)]
            for j_off, c in enumerate(cols_of[r]):
                j = int(crow[r]) + j_off  # global block index into w[nnz]
                per_g[int(gof[c])].append((int(chunkof[c]), j))
            for g in range(4):
                per_g[g].sort()  # ascending chunk -> progressive DMA deps
            chains[r] = per_g
        return chains

    # pass 1: degree-ordered groups -> provisional loads -> quads
    gof1, chunkof1 = assign_groups(np.argsort(-deg, kind="stable"))
    chains1 = build_chains(gof1, chunkof1)
    loads = {
        int(r): np.array([max(1, len(chains1[int(r)][g])) for g in range(4)])
        for r in rows
    }
    rows_left = sorted((int(r) for r in rows), key=lambda r: -int(loads[r].max()))
    quad_rows = [[0] * 4 for _ in range(N_QUADS)]
    quad_load = [np.zeros(4, np.int64) for _ in range(N_QUADS)]
    quad_fill = [0] * N_QUADS
    for r in rows_left:
        cands = [q for q in range(N_QUADS) if quad_fill[q] < 4]
        q = min(cands, key=lambda qi: int((quad_load[qi] + loads[r]).max()))
        quad_rows[q][quad_fill[q]] = r
        quad_load[q] += loads[r]
        quad_fill[q] += 1

    # pass 2: first-use chunk order given the quad schedule
    seen = set()
    order2 = []
    for q in range(N_QUADS):
        for rl in range(4):
            for c in cols_of[quad_rows[q][rl]]:
                c = int(c)
                if c not in seen:
                    seen.add(c)
                    order2.append(c)
    for c in range(CB):
        if c not in seen:
            order2.append(c)
    gof, chunkof = assign_groups(order2)
    chains = build_chains(gof, chunkof)

    # dummy (zero-weight) matmuls for empty chains: use the group's chunk 0
    g_chunk0_col = {}
    for c in range(CB):
        g = int(gof[c])
        if int(chunkof[c]) == 0:
            g_chunk0_col[g] = 0

    slot_counter = [1] * 4  # slot 0 per group = zero weights (dummies)
    sched = []
    for q in range(N_QUADS):
        todo = {}
        for rl in range(4):
            r = quad_rows[q][rl]
            for g in range(4):
                ch = list(chains[r][g])
                if not ch:
                    ch = [(0, -1)]  # dummy on chunk 0 of this group
                todo[(rl, g)] = ch
        max_len = max(len(v) for v in todo.values())
        items = []
        for k in range(max_len):
            for rl in range(4):
                for g in range(4):
                    ch = todo[(rl, g)]
                    if k < len(ch):
                        chunk, widx = ch[k]
                        if widx < 0:
                            slot = 0
                        else:
                            slot = slot_counter[g]
                            slot_counter[g] += 1
                        items.append(
                            (rl, g, chunk, slot, widx, k == 0, k == len(ch) - 1)
                        )
        sched.append(items)
    n_slots = max(slot_counter)
    return gof, chunkof, sched, n_slots, quad_rows


def _build_half(sched, n_slots, quad_rows, reps=1, ablate="", n_xp=8, n_wp=8,
                out_group=2):
    nc = bacc.Bacc(
        "TRN2",
        target_bir_lowering=False,
        debug=False,
        num_devices=4,
    )
    dt = mybir.dt.bfloat16
    f32 = mybir.dt.float32

    xt = nc.dram_tensor("xt", [128, N_CHUNKS * BQ], dt, kind="ExternalInput")
    wt = nc.dram_tensor("wt", [128, n_slots * BS], dt, kind="ExternalInput")
    # per-quad per-partition bias column (quad q, partition p = out feature
    # quad_rows[q][p//32]*32 + p%32), added by the ACT evac copy
    bias_d = nc.dram_tensor(
        "bias_d", [128, N_QUADS], mybir.dt.float32, kind="ExternalInput"
    )
    yt = nc.dram_tensor("yt", [128, N_QUADS * BQ], dt, kind="ExternalOutput")

    hoist = "hoist_in" in ablate
    with TileContext(nc) as tc:
        with (
            tc.tile_pool(
                name="persist", bufs=(2 if reps > 1 and not hoist else 1)
            ) as persist,
            tc.tile_pool(name="psum", bufs=2, space="PSUM") as psum_pool,
            tc.tile_pool(name="stage", bufs=3) as stage_pool,
            tc.tile_pool(name="tmp", bufs=2) as tmp_pool,
        ):
          def load_inputs():
                # bias first (tiny), then wt/xt pieces interleaved so early
                # quads unblock while later pieces stream.
                bias_sb = persist.tile(
                    [128, N_QUADS], mybir.dt.float32, tag="bias_sb", name="bias_sb"
                )
                nc.sync.dma_start(out=bias_sb, in_=bias_d[:])
                cpp = N_CHUNKS // n_xp  # chunks per xt piece
                sp = -(-n_slots // n_wp)  # slots per wt piece
                xt_pieces = [None] * n_xp
                wt_pieces = [None] * n_wp
                for p in range(max(n_xp, n_wp)):
                    if p < n_wp:
                        lo = p * sp * BS
                        hi = min((p + 1) * sp * BS, n_slots * BS)
                        if lo < hi:
                            wp = persist.tile(
                                [128, hi - lo], dt, tag=f"wt_sb{p}",
                                name=f"wt_sb{p}",
                            )
                            nc.sync.dma_start(out=wp, in_=wt[:, lo:hi])
                            wt_pieces[p] = wp
                    if p < n_xp:
                        xp = persist.tile(
                            [128, cpp * BQ], dt, tag=f"xt_sb{p}",
                            name=f"xt_sb{p}",
                        )
                        nc.sync.dma_start(
                            out=xp, in_=xt[:, p * cpp * BQ : (p + 1) * cpp * BQ]
                        )
                        xt_pieces[p] = xp
                return xt_pieces, cpp, bias_sb, wt_pieces, sp

          if hoist:
              xt_pieces, cpp, bias_sb, wt_pieces, sp = load_inputs()
          for _rep in range(reps):
            if not hoist:
                xt_pieces, cpp, bias_sb, wt_pieces, sp = load_inputs()

            pending = None
            stage = None

            def emit_evac(q, P):
                nonlocal stage
                if q % out_group == 0:
                    stage = stage_pool.tile(
                        [128, out_group * BQ], dt, tag="st", name=f"st_{q}"
                    )
                dst = stage[:, (q % out_group) * BQ : (q % out_group + 1) * BQ]
                e0 = tmp_pool.tile([128, BQ], dt, tag="e0", name=f"e0_{q}")
                e1 = tmp_pool.tile([128, BQ], dt, tag="e1", name=f"e1_{q}")
                e2 = tmp_pool.tile([128, BQ], dt, tag="e2", name=f"e2_{q}")
                d0 = tmp_pool.tile([128, BQ], dt, tag="d0", name=f"d0_{q}")
                d1 = tmp_pool.tile([128, BQ], dt, tag="d1", name=f"d1_{q}")
                # Free all 4 psum banks ASAP (they gate quad q+2's matmuls):
                # DVE drains P2 then P3, ACT drains P0 (bias rides the
                # Identity) and P1 in parallel; cross-adds follow on DVE/POOL.
                # ACT (2x-mode psum reads) drains P0-P2, bias riding the
                # Identity; DVE fuses the P3 read into its first add and
                # owns the reduction tree.  GPSIMD is much slower per op
                # than its spec sheet -- keep it out of the pipeline.
                nc.scalar.activation(
                    e0,
                    P[0][:, :],
                    mybir.ActivationFunctionType.Identity,
                    bias=bias_sb[:, q : q + 1],
                )
                nc.scalar.copy(e1, P[1][:, :])
                nc.scalar.copy(e2, P[2][:, :])
                nc.vector.tensor_add(d1, e2, P[3][:, :])
                if "evac_l1" in ablate:
                    nc.vector.tensor_copy(dst, d1)
                else:
                    nc.vector.tensor_add(d0, e0, e1)
                    nc.vector.tensor_add(dst, d0, d1)
                if q % out_group == out_group - 1:
                    q0 = q - (out_group - 1)
                    nc.scalar.dma_start(
                        out=yt[:, q0 * BQ : (q0 + out_group) * BQ], in_=stage
                    )

            for q, items in enumerate(sched):
                psums = (
                    []
                    if "no_mm" in ablate
                    else [
                        psum_pool.tile(
                            [128, BQ], f32, tag=f"acc{g}", name=f"acc{g}_{_rep}_{q}"
                        )
                        for g in range(4)
                    ]
                )
                for (rl, g, chunk, slot, widx, start, stop) in items:
                    if "no_mm" in ablate:
                        break
                    dst_ap = psums[g][rl * BS : (rl + 1) * BS, :]
                    pc, ps = divmod(slot, sp)
                    xc, xo = divmod(chunk, cpp)
                    nc.tensor.matmul(
                        out=dst_ap,
                        lhsT=wt_pieces[pc][
                            g * BS : (g + 1) * BS, ps * BS : (ps + 1) * BS
                        ],
                        rhs=xt_pieces[xc][
                            g * BS : (g + 1) * BS, xo * BQ : (xo + 1) * BQ
                        ],
                        start=start,
                        stop=stop,
                        tile_position=(g * BS, rl * BS),
                    )
                if "no_evac" in ablate:
                    continue
                if pending is not None:
                    emit_evac(*pending)
                pending = (q, psums)
            if pending is not None:
                emit_evac(*pending)
            pending = None
            if "no_evac" in ablate:
                stq = stage_pool.tile(
                    [128, out_group * BQ], dt, tag="st", name=f"stq_{_rep}"
                )
                nc.vector.memset(stq, 0.0)
                for q in range(0, N_QUADS, out_group):
                    nc.scalar.dma_start(
                        out=yt[:, q * BQ : (q + out_group) * BQ], in_=stq
                    )
    nc.compile()
    _strip_mm_incs(nc)
    return nc


def _pack_half(x_quarter, gof, chunkof, sched, n_slots, quad_rows, w, bias):
    import ml_dtypes

    bf16 = ml_dtypes.bfloat16
    xt_in = np.zeros((128, N_CHUNKS * BQ), np.float32)
    for c in range(CB):
        g = int(gof[c])
        ch = int(chunkof[c])
        xt_in[g * BS : (g + 1) * BS, ch * BQ : (ch + 1) * BQ] = x_quarter[
            :, c * BS : (c + 1) * BS
        ].T
    xt_in = np.ascontiguousarray(xt_in.astype(bf16))

    wt_in = np.zeros((128, n_slots * BS), np.float32)
    for items in sched:
        for (_rl, g, _chunk, slot, widx, _a, _b) in items:
            if widx >= 0:
                wt_in[g * BS : (g + 1) * BS, slot * BS : (slot + 1) * BS] = w[
                    widx
                ].T
    wt_in = np.ascontiguousarray(wt_in.astype(bf16))

    # per-quad bias column: partition p = out feature quad_rows[q][p//32]+p%32
    bias_in = np.zeros((128, N_QUADS), np.float32)
    for q in range(N_QUADS):
        for rl in range(4):
            r = quad_rows[q][rl]
            bias_in[rl * BS : (rl + 1) * BS, q] = bias[r * BS : (r + 1) * BS]
    bias_in = np.ascontiguousarray(bias_in)
    return xt_in, wt_in, bias_in


def _prepare(crow, col):
    key = (crow.tobytes(), col.tobytes())
    if key not in _CACHE:
        rows_a = np.arange(RBH)
        rows_b = np.arange(RBH, RB)
        la = _layout_half(crow, col, rows_a)
        lb = _layout_half(crow, col, rows_b)
        nca = _build_half(la[2], la[3], la[4])
        ncb = _build_half(lb[2], lb[3], lb[4])
        _CACHE.clear()
        _CACHE[key] = (la, lb, nca, ncb)
    return _CACHE[key]


def kernel(x, crow_indices, col_indices, mask, weight, bias):
    x = np.asarray(x, np.float32)
    crow = np.asarray(crow_indices, np.int64)
    col = np.asarray(col_indices, np.int64)
    w = np.asarray(mask, np.float32) * np.asarray(weight, np.float32)
    bias_np = np.asarray(bias, np.float32)

    la, lb, nca, ncb = _prepare(crow, col)

    in_a, in_b = [], []
    for q in range(4):
        xq = x[q * BQ : (q + 1) * BQ]
        gof, chunkof, sched, n_slots, quad_rows = la
        xt, wt, bi = _pack_half(xq, gof, chunkof, sched, n_slots, quad_rows, w, bias_np)
        in_a.append({"xt": xt, "wt": wt, "bias_d": bi})
        gof, chunkof, sched, n_slots, quad_rows = lb
        xt, wt, bi = _pack_half(xq, gof, chunkof, sched, n_slots, quad_rows, w, bias_np)
        in_b.append({"xt": xt, "wt": wt, "bias_d": bi})

    res_a = run_bass_kernel_spmd(nca, in_a, core_ids=list(range(4)))
    res_b = run_bass_kernel_spmd(ncb, in_b, core_ids=list(range(4)))

    y = np.empty((BATCH, RB * BS), np.float32)
    for half, (layout, res, row0) in enumerate(
        ((la, res_a, 0), (lb, res_b, RBH))
    ):
        gof, chunkof, sched, n_slots, quad_rows = layout
        for q4 in range(4):
            ytc = np.asarray(res.results[q4]["yt"]).astype(np.float32)
            for q in range(N_QUADS):
                for rl in range(4):
                    r = quad_rows[q][rl]
                    y[
                        q4 * BQ : (q4 + 1) * BQ, r * BS : (r + 1) * BS
                    ] = ytc[rl * BS : (rl + 1) * BS, q * BQ : (q + 1) * BQ].T
    return np.ascontiguousarray(y)


# revision 17
# speedup vs baseline: 4.5049x; 1.0224x over previous
"""Block-sparse linear (BSR 32x32) on 8 NeuronCores: 4-way batch x 2-way
row sharding, N=512 matmuls.

Per-instruction PE cost (~14.5 ns) dominates this kernel, so the win over
v1 is halving instructions per core: batch quarters (BPC=512) double the
moving dim per matmul at equal per-core DMA, and row halves (64 row blocks
per core) halve the block count per core.

Cores 0-3: batch quarter q = core, row blocks 0-63  (program A)
Cores 4-7: batch quarter q = core-4, row blocks 64-127 (program B)
Host assembles y from the two halves.  A and B have different sparsity ->
two compiled programs, dispatched as two 4-core SPMD jobs.
"""

import sys

import numpy as np

sys.path.insert(0, "/opt/trn_rl_repo")

import concourse.mybir as mybir
from concourse import bacc
from concourse.bass_utils import run_bass_kernel_spmd
from concourse.tile import TileContext

BS = 32
RB = 128
CB = 128
BATCH = 2048
N_CORES = 8
BQ = 512          # batch per core (quarter)
RBH = 64          # row blocks per half
N_CHUNKS = CB // 4
N_QUADS = RBH // 4

_CACHE = {}


def _strip_mm_incs(nc):
    """See kernel.py: drop per-matmul sem increments nobody waits on."""
    import concourse.mybir as mb

    blocks = nc.m.functions[0].blocks
    all_insts = [i for blk in blocks for i in blk.instructions]
    mms = [
        i
        for i in all_insts
        if isinstance(i, mb.InstMatmult) and i.sync_info is not None
    ]
    sem_ids = set()
    for m in mms:
        for u in m.sync_info.on_update:
            sem_ids.add(u.id)
    if not sem_ids:
        return
    assert len(sem_ids) == 1, f"multiple PE sems: {sem_ids}"
    sem = sem_ids.pop()
    for inst in all_insts:
        if isinstance(inst, mb.InstMatmult) or inst.sync_info is None:
            continue
        for u in inst.sync_info.on_update:
            assert u.id != sem, f"non-MM update to PE sem by {inst.name}"
    waits = []
    for inst in all_insts:
        si = inst.sync_info
        if si is None:
            continue
        for wi, w in enumerate(si.on_wait):
            if w.id == sem:
                assert w.wait_mode == "sem-ge-imm", w
                waits.append((inst, wi, w.wait_value))
    thresholds = sorted({v for (_, _, v) in waits})
    rank = {v: i + 1 for i, v in enumerate(thresholds)}
    keep = set(thresholds)
    k = 0
    for m in mms:
        k += 1
        if k not in keep:
            m.sync_info = mb.SyncInfo(
                on_wait=list(m.sync_info.on_wait), on_update=[]
            )
    for inst, wi, v in waits:
        si = inst.sync_info
        new_waits = list(si.on_wait)
        w = new_waits[wi]
        new_waits[wi] = mb.SyncWait(
            sync_type=w.sync_type,
            id=w.id,
            ant_name=w.ant_name,
            wait_mode=w.wait_mode,
            wait_value=rank[v],
            wait_reg=None,
        )
        inst.sync_info = mb.SyncInfo(
            on_wait=new_waits, on_update=list(si.on_update)
        )


def _layout_half(crow, col, rows):
    """Layout for one row half: col -> (group, chunk), chains, quad schedule.

    Same PSUM discipline as v1: per quad, four psum banks (one per
    partition group g); the chain of row r's blocks with col-group g
    accumulates serially on PE tile (g, rl).

    Two passes: pass 1 assigns groups by degree to balance lanes and picks
    quads; pass 2 re-numbers chunks in FIRST-USE order (by quad) so early
    quads depend only on early xt pieces and the PE starts while xt is
    still streaming.  Bias rides the evacuation (ACT bias operand), not
    matmuls; empty (row, group) chains get a zero-weight dummy matmul so
    every psum region is written.
    """
    cols_of = {int(r): col[int(crow[r]) : int(crow[r + 1])].astype(int) for r in rows}
    deg = np.zeros(CB, np.int64)
    for r in rows:
        for c in cols_of[int(r)]:
            deg[c] += 1

    def assign_groups(order):
        load = [0] * 4
        cnt = [0] * 4
        gof = np.zeros(CB, np.int64)
        chunkof = np.zeros(CB, np.int64)
        for c in order:
            cands = [g for g in range(4) if cnt[g] < N_CHUNKS]
            g = min(cands, key=lambda gi: load[gi])
            gof[c] = g
            chunkof[c] = cnt[g]
            load[g] += int(deg[c])
            cnt[g] += 1
        return gof, chunkof

    def build_chains(gof, chunkof):
        chains = {}
        for r in rows:
            r = int(r)
            per_g = [[] for _ in range(4)]
            for j_off, c in enumerate(cols_of[r]):
                j = int(crow[r]) + j_off  # global block index into w[nnz]
                per_g[int(gof[c])].append((int(chunkof[c]), j))
            for g in range(4):
                per_g[g].sort()  # ascending chunk -> progressive DMA deps
            chains[r] = per_g
        return chains

    # pass 1: degree-ordered groups -> provisional loads -> quads
    gof1, chunkof1 = assign_groups(np.argsort(-deg, kind="stable"))
    chains1 = build_chains(gof1, chunkof1)
    loads = {
        int(r): np.array([max(1, len(chains1[int(r)][g])) for g in range(4)])
        for r in rows
    }
    rows_left = sorted((int(r) for r in rows), key=lambda r: -int(loads[r].max()))
    quad_rows = [[0] * 4 for _ in range(N_QUADS)]
    quad_load = [np.zeros(4, np.int64) for _ in range(N_QUADS)]
    quad_fill = [0] * N_QUADS
    for r in rows_left:
        cands = [q for q in range(N_QUADS) if quad_fill[q] < 4]
        q = min(cands, key=lambda qi: int((quad_load[qi] + loads[r]).max()))
        quad_rows[q][quad_fill[q]] = r
        quad_load[q] += loads[r]
        quad_fill[q] += 1

    # pass 2: first-use chunk order given the quad schedule
    seen = set()
    order2 = []
    for q in range(N_QUADS):
        for rl in range(4):
            for c in cols_of[quad_rows[q][rl]]:
                c = int(c)
                if c not in seen:
                    seen.add(c)
                    order2.append(c)
    for c in range(CB):
        if c not in seen:
            order2.append(c)
    gof, chunkof = assign_groups(order2)
    chains = build_chains(gof, chunkof)

    # dummy (zero-weight) matmuls for empty chains: use the group's chunk 0
    g_chunk0_col = {}
    for c in range(CB):
        g = int(gof[c])
        if int(chunkof[c]) == 0:
            g_chunk0_col[g] = 0

    slot_counter = [1] * 4  # slot 0 per group = zero weights (dummies)
    sched = []
    for q in range(N_QUADS):
        todo = {}
        for rl in range(4):
            r = quad_rows[q][rl]
            for g in range(4):
                ch = list(chains[r][g])
                if not ch:
                    ch = [(0, -1)]  # dummy on chunk 0 of this group
                todo[(rl, g)] = ch
        max_len = max(len(v) for v in todo.values())
        items = []
        for k in range(max_len):
            for rl in range(4):
                for g in range(4):
                    ch = todo[(rl, g)]
                    if k < len(ch):
                        chunk, widx = ch[k]
                        if widx < 0:
                            slot = 0
                        else:
                            slot = slot_counter[g]
                            slot_counter[g] += 1
                        items.append(
                            (rl, g, chunk, slot, widx, k == 0, k == len(ch) - 1)
                        )
        sched.append(items)
    n_slots = max(slot_counter)
    return gof, chunkof, sched, n_slots, quad_rows


def _build_half(sched, n_slots, quad_rows, reps=1, ablate="", n_xp=8, n_wp=8,
                out_group=2):
    nc = bacc.Bacc(
        "TRN2",
        target_bir_lowering=False,
        debug=False,
        num_devices=4,
    )
    dt = mybir.dt.bfloat16
    f32 = mybir.dt.float32

    xt = nc.dram_tensor("xt", [128, N_CHUNKS * BQ], dt, kind="ExternalInput")
    wt = nc.dram_tensor("wt", [128, n_slots * BS], dt, kind="ExternalInput")
    # per-quad per-partition bias column (quad q, partition p = out feature
    # quad_rows[q][p//32]*32 + p%32), added by the ACT evac copy
    bias_d = nc.dram_tensor(
        "bias_d", [128, N_QUADS], mybir.dt.float32, kind="ExternalInput"
    )
    yt = nc.dram_tensor("yt", [128, N_QUADS * BQ], dt, kind="ExternalOutput")

    hoist = "hoist_in" in ablate
    with TileContext(nc) as tc:
        with (
            tc.tile_pool(
                name="persist", bufs=(2 if reps > 1 and not hoist else 1)
            ) as persist,
            tc.tile_pool(name="psum", bufs=2, space="PSUM") as psum_pool,
            tc.tile_pool(name="stage", bufs=3) as stage_pool,
            tc.tile_pool(name="tmp", bufs=4) as tmp_pool,
        ):
          def load_inputs():
                # bias first (tiny), then wt/xt pieces interleaved so early
                # quads unblock while later pieces stream.
                bias_sb = persist.tile(
                    [128, N_QUADS], mybir.dt.float32, tag="bias_sb", name="bias_sb"
                )
                nc.sync.dma_start(out=bias_sb, in_=bias_d[:])
                cpp = N_CHUNKS // n_xp  # chunks per xt piece
                sp = -(-n_slots // n_wp)  # slots per wt piece
                xt_pieces = [None] * n_xp
                wt_pieces = [None] * n_wp
                for p in range(max(n_xp, n_wp)):
                    if p < n_wp:
                        lo = p * sp * BS
                        hi = min((p + 1) * sp * BS, n_slots * BS)
                        if lo < hi:
                            wp = persist.tile(
                                [128, hi - lo], dt, tag=f"wt_sb{p}",
                                name=f"wt_sb{p}",
                            )
                            nc.sync.dma_start(out=wp, in_=wt[:, lo:hi])
                            wt_pieces[p] = wp
                    if p < n_xp:
                        xp = persist.tile(
                            [128, cpp * BQ], dt, tag=f"xt_sb{p}",
                            name=f"xt_sb{p}",
                        )
                        nc.sync.dma_start(
                            out=xp, in_=xt[:, p * cpp * BQ : (p + 1) * cpp * BQ]
                        )
                        xt_pieces[p] = xp
                return xt_pieces, cpp, bias_sb, wt_pieces, sp

          if hoist:
              xt_pieces, cpp, bias_sb, wt_pieces, sp = load_inputs()
          for _rep in range(reps):
            if not hoist:
                xt_pieces, cpp, bias_sb, wt_pieces, sp = load_inputs()

            pending = None
            stage = None

            def emit_evac(q, P):
                nonlocal stage
                if q % out_group == 0:
                    stage = stage_pool.tile(
                        [128, out_group * BQ], dt, tag="st", name=f"st_{q}"
                    )
                dst = stage[:, (q % out_group) * BQ : (q % out_group + 1) * BQ]
                e0 = tmp_pool.tile([128, BQ], dt, tag="e0", name=f"e0_{q}")
                e1 = tmp_pool.tile([128, BQ], dt, tag="e1", name=f"e1_{q}")
                e2 = tmp_pool.tile([128, BQ], dt, tag="e2", name=f"e2_{q}")
                d0 = tmp_pool.tile([128, BQ], dt, tag="d0", name=f"d0_{q}")
                d1 = tmp_pool.tile([128, BQ], dt, tag="d1", name=f"d1_{q}")
                # Free all 4 psum banks ASAP (they gate quad q+2's matmuls):
                # DVE drains P2 then P3, ACT drains P0 (bias rides the
                # Identity) and P1 in parallel; cross-adds follow on DVE/POOL.
                # DVE drains P2 then P3 (shortest path to freeing the
                # banks that gate quad q+2); ACT drains P0 (bias rides the
                # Identity) and P1 in parallel.  GPSIMD is much slower per
                # op than its spec sheet -- keep it out of the pipeline.
                nc.vector.tensor_copy(e2, P[2][:, :])
                nc.vector.tensor_add(d1, e2, P[3][:, :])
                nc.scalar.activation(
                    e0,
                    P[0][:, :],
                    mybir.ActivationFunctionType.Identity,
                    bias=bias_sb[:, q : q + 1],
                )
                nc.scalar.copy(e1, P[1][:, :])
                if "evac_l1" in ablate:
                    nc.vector.tensor_copy(dst, d1)
                else:
                    nc.vector.tensor_add(d0, e0, e1)
                    nc.vector.tensor_add(dst, d0, d1)
                if q % out_group == out_group - 1:
                    q0 = q - (out_group - 1)
                    nc.scalar.dma_start(
                        out=yt[:, q0 * BQ : (q0 + out_group) * BQ], in_=stage
                    )

            for q, items in enumerate(sched):
                psums = (
                    []
                    if "no_mm" in ablate
                    else [
                        psum_pool.tile(
                            [128, BQ], f32, tag=f"acc{g}", name=f"acc{g}_{_rep}_{q}"
                        )
                        for g in range(4)
                    ]
                )
                for (rl, g, chunk, slot, widx, start, stop) in items:
                    if "no_mm" in ablate:
                        break
                    dst_ap = psums[g][rl * BS : (rl + 1) * BS, :]
                    pc, ps = divmod(slot, sp)
                    xc, xo = divmod(chunk, cpp)
                    nc.tensor.matmul(
                        out=dst_ap,
                        lhsT=wt_pieces[pc][
                            g * BS : (g + 1) * BS, ps * BS : (ps + 1) * BS
                        ],
                        rhs=xt_pieces[xc][
                            g * BS : (g + 1) * BS, xo * BQ : (xo + 1) * BQ
                        ],
                        start=start,
                        stop=stop,
                        tile_position=(g * BS, rl * BS),
                    )
                if "no_evac" in ablate:
                    continue
                if pending is not None:
                    emit_evac(*pending)
                pending = (q, psums)
            if pending is not None:
                emit_evac(*pending)
            pending = None
            if "no_evac" in ablate:
                stq = stage_pool.tile(
                    [128, out_group * BQ], dt, tag="st", name=f"stq_{_rep}"
                )
                nc.vector.memset(stq, 0.0)
                for q in range(0, N_QUADS, out_group):
                    nc.scalar.dma_start(
                        out=yt[:, q * BQ : (q + out_group) * BQ], in_=stq
                    )
    nc.compile()
    _strip_mm_incs(nc)
    return nc


def _pack_half(x_quarter, gof, chunkof, sched, n_slots, quad_rows, w, bias):
    import ml_dtypes

    bf16 = ml_dtypes.bfloat16
    xt_in = np.zeros((128, N_CHUNKS * BQ), np.float32)
    for c in range(CB):
        g = int(gof[c])
        ch = int(chunkof[c])
        xt_in[g * BS : (g + 1) * BS, ch * BQ : (ch + 1) * BQ] = x_quarter[
            :, c * BS : (c + 1) * BS
        ].T
    xt_in = np.ascontiguousarray(xt_in.astype(bf16))

    wt_in = np.zeros((128, n_slots * BS), np.float32)
    for items in sched:
        for (_rl, g, _chunk, slot, widx, _a, _b) in items:
            if widx >= 0:
                wt_in[g * BS : (g + 1) * BS, slot * BS : (slot + 1) * BS] = w[
                    widx
                ].T
    wt_in = np.ascontiguousarray(wt_in.astype(bf16))

    # per-quad bias column: partition p = out feature quad_rows[q][p//32]+p%32
    bias_in = np.zeros((128, N_QUADS), np.float32)
    for q in range(N_QUADS):
        for rl in range(4):
            r = quad_rows[q][rl]
            bias_in[rl * BS : (rl + 1) * BS, q] = bias[r * BS : (r + 1) * BS]
    bias_in = np.ascontiguousarray(bias_in)
    return xt_in, wt_in, bias_in


def _prepare(crow, col):
    key = (crow.tobytes(), col.tobytes())
    if key not in _CACHE:
        rows_a = np.arange(RBH)
        rows_b = np.arange(RBH, RB)
        la = _layout_half(crow, col, rows_a)
        lb = _layout_half(crow, col, rows_b)
        nca = _build_half(la[2], la[3], la[4])
        ncb = _build_half(lb[2], lb[3], lb[4])
        _CACHE.clear()
        _CACHE[key] = (la, lb, nca, ncb)
    return _CACHE[key]


def kernel(x, crow_indices, col_indices, mask, weight, bias):
    x = np.asarray(x, np.float32)
    crow = np.asarray(crow_indices, np.int64)
    col = np.asarray(col_indices, np.int64)
    w = np.asarray(mask, np.float32) * np.asarray(weight, np.float32)
    bias_np = np.asarray(bias, np.float32)

    la, lb, nca, ncb = _prepare(crow, col)

    in_a, in_b = [], []
    for q in range(4):
        xq = x[q * BQ : (q + 1) * BQ]
        gof, chunkof, sched, n_slots, quad_rows = la
        xt, wt, bi = _pack_half(xq, gof, chunkof, sched, n_slots, quad_rows, w, bias_np)
        in_a.append({"xt": xt, "wt": wt, "bias_d": bi})
        gof, chunkof, sched, n_slots, quad_rows = lb
        xt, wt, bi = _pack_half(xq, gof, chunkof, sched, n_slots, quad_rows, w, bias_np)
        in_b.append({"xt": xt, "wt": wt, "bias_d": bi})

    res_a = run_bass_kernel_spmd(nca, in_a, core_ids=list(range(4)))
    res_b = run_bass_kernel_spmd(ncb, in_b, core_ids=list(range(4)))

    y = np.empty((BATCH, RB * BS), np.float32)
    for half, (layout, res, row0) in enumerate(
        ((la, res_a, 0), (lb, res_b, RBH))
    ):
        gof, chunkof, sched, n_slots, quad_rows = layout
        for q4 in range(4):
            ytc = np.asarray(res.results[q4]["yt"]).astype(np.float32)
            for q in range(N_QUADS):
                for rl in range(4):
                    r = quad_rows[q][rl]
                    y[
                        q4 * BQ : (q4 + 1) * BQ, r * BS : (r + 1) * BS
                    ] = ytc[rl * BS : (rl + 1) * BS, q * BQ : (q + 1) * BQ].T
    return np.ascontiguousarray(y)
